# revision 27
# baseline (speedup 1.0000x reference)
"""Trainium2 Bass kernel for BiDAF-style bidirectional attention.

Reference computation (per batch element n; M=1 folded away):
    s[i,j]  = h[i].w_h + u[j].w_u + (h[i]*u[j]).w_hu + b      [JX, JQ]
    a_u     = softmax_j(s);     u_a[i] = sum_j a_u[i,j] u[j]   (c2q)
    a_h     = softmax_i(max_j s);  h_a = sum_i a_h[i] h[i]     (q2c)
    out     = concat(h, u_a, h*u_a, h*h_a)                     [JX, 4D]

Sharding: data-parallel over batch N=8, one NeuronCore per batch element.
alpha_b drops out entirely (both softmaxes are shift-invariant).

v8 structure (51us f32 baseline):
  - outputs split by layout: out0 f32 [JX, D] = h (one DRAM->DRAM DMA, gated
    to run after the loads: its big descriptors win the SDMA per-packet
    round-robin and would starve them); out12 bf16 [JX, 2D] = u_a | h*u_a;
    o4T bf16 [D, JX] = (h*h_a) TRANSPOSED.  The host stitches/transposes.
    bf16 stores halve traffic; the 2e-2 rel-err gate leaves ~5x margin.
  - o4T trick: h_a is per-PARTITION in the hT layout, so h*h_a is four
    all-bf16 tensor_scalar ops over [128, JX] with no PSUM broadcast, no
    bc matmul, and no SBUF broadcast copy.
  - bf16 matmul operands (hT, ET, u, uw'); f32 accumulation in PSUM.
  - few big DMAs; h loads as 8 singles so completions stagger and the
    transpose pipeline chases them.
  - PE spine: t0..t3+uwT transposes -> scores-b0 -> t4..t7 -> etr-b0 ->
    scores-b1 -> c2q ups interleaved with etr-b1/hap -> q2c chain.
  - elementwise tail: col1 = up*zr (ACT, PSUM-read), col2 = (up*zr)*h as
    f32-in DVE stt (mixed bf16xf32 tensor_tensor on DVE is a 4x-slow
    path - avoided), o4T on DVE/GpSimd.
"""

import numpy as np

N_B, M_B, JX, JQ, D = 8, 1, 1024, 128, 512
P = 128
NT = JX // P   # 8 i-tiles
KC = D // P    # 4 d-chunks
IB = 512       # i-block width for score matmuls
NB = JX // IB  # 2 blocks
TPB = NT // NB  # tiles per block

_CACHE = {}


def _build_program():
    from contextlib import ExitStack

    import concourse.bass as bass
    import concourse.tile as tile
    from concourse import bacc, mybir
    from concourse.masks import make_identity
    from concourse.tile_rust import add_dep_helper

    f32 = mybir.dt.float32
    f32r = mybir.dt.float32r
    bf16 = mybir.dt.bfloat16
    EXP = mybir.ActivationFunctionType.Exp
    AX = mybir.AxisListType.X
    MUL = mybir.AluOpType.mult
    ds = bass.ds

    nc = bacc.Bacc("TRN2", target_bir_lowering=False, debug=False, num_devices=8)
    h_d = nc.dram_tensor("h", [JX, D], f32, kind="ExternalInput").ap()
    u_d = nc.dram_tensor("u", [JQ, D], f32, kind="ExternalInput").ap()
    aw_d = nc.dram_tensor("alpha_w", [3 * D], f32, kind="ExternalInput").ap()
    out0_d = nc.dram_tensor("out0", [JX, D], f32, kind="ExternalOutput").ap()
    out12_d = nc.dram_tensor("out12", [JX, 2 * D], bf16, kind="ExternalOutput").ap()
    o4T_d = nc.dram_tensor("o4T", [D, JX], bf16, kind="ExternalOutput").ap()

    with tile.TileContext(nc) as tc, ExitStack() as ctx:
        consts = ctx.enter_context(tc.tile_pool(name="consts", bufs=1))
        stage = ctx.enter_context(tc.tile_pool(name="stage", bufs=4))
        # PSUM budget (8 banks): tp=2, s0=2, ua=2, acc=1, hap=1
        ps = ctx.enter_context(tc.tile_pool(name="ps", bufs=2, space="PSUM"))

        # ---- PE warmup: opens the HAM clock-gate (1.2 -> 2.4 GHz) while
        # the first h loads stream in.
        warm_f = consts.tile([P, D], f32)
        nc.vector.memset(warm_f[:], 0.25)
        warm = consts.tile([P, D], f32r)
        nc.vector.tensor_copy(warm[:], warm_f[:])
        wp = ps.tile([P, D], f32, tag="acc", bufs=1)
        for w in range(2):
            nc.tensor.matmul(
                wp[:], warm[:, ds(0, P)], warm[:], start=True, stop=True,
            )

        # ---- constants ----
        ident_f = consts.tile([P, P], f32)
        make_identity(nc, ident_f[:])
        ident = consts.tile([P, P], f32r)
        nc.vector.tensor_copy(ident[:], ident_f[:])
        ident16 = consts.tile([P, P], bf16)
        nc.vector.tensor_copy(ident16[:], ident_f[:])
        ones_row_f = consts.tile([1, P], f32)
        nc.vector.memset(ones_row_f[:], 1.0)
        ones_row = consts.tile([1, P], f32r)
        nc.scalar.copy(ones_row[:], ones_row_f[:])
        ones_col = consts.tile([P, 1], f32)
        nc.vector.memset(ones_col[:], 1.0)

        # ---- loads (sync queue): h0, h1 first (they gate the transpose
        # pipeline), then aw + u (uw' prep chain), then h2-h7.
        h_all = consts.tile([P, NT * D], f32r)    # tile t: h[t*128+p, d]
        h_f = h_all[:].bitcast(f32)
        for t in range(2):
            nc.sync.dma_start(
                h_all[:, ds(t * D, D)], h_d[ds(t * P, P), :].bitcast(f32r)
            )
        aw_sb = consts.tile([1, 3 * D], f32r)
        nc.sync.dma_start(aw_sb[:], aw_d.rearrange("(o d) -> o d", o=1).bitcast(f32r))
        u_sb = consts.tile([JQ, D], f32r)
        nc.sync.dma_start(u_sb[:], u_d[:].bitcast(f32r))
        u_f = u_sb[:].bitcast(f32)
        last_h = None
        for t in range(2, NT):
            last_h = nc.sync.dma_start(
                h_all[:, ds(t * D, D)], h_d[ds(t * P, P), :].bitcast(f32r)
            )

        # passthrough column: DRAM->DRAM copy of h in 8KB rows, gated out of
        # the load window (see docstring).
        d2d = nc.sync.dma_start(
            out0_d.rearrange("(a b) d -> a (b d)", b=4),
            h_d.rearrange("(a b) d -> a (b d)", b=4),
        )
        add_dep_helper(d2d.ins, last_h.ins, sync=True,
                       reason="keep d2d pass copy out of the load window")

        u16 = consts.tile([JQ, D], bf16)
        nc.gpsimd.tensor_copy(u16[:], u_f)

        # alpha_w partition-broadcast on-chip: K=1 matmuls into PSUM, read
        # directly by the DVE.  whu first (gates uw').  wu goes to the "ua"
        # slot so it doesn't wait for whu's bank to free (that wait jammed
        # the PE FIFO ahead of the t2/t3 transposes).
        def wcast(c, tag):
            wt = ps.tile([P, D], f32, tag=tag)
            nc.tensor.matmul(
                wt[:], ones_row[:], aw_sb[:, ds(c * D, D)], start=True, stop=True
            )
            return wt

        # ---- hT via PE transposes (f32r ident moving), cast-evict to bf16.
        hT16 = consts.tile([P, KC * JX], bf16)    # chunk k: hT[k*128+p, i]
        hT3 = hT16[:].rearrange("p (k x) -> p k x", k=KC)

        def transpose_tile(t):
            pt = ps.tile([P, KC * P], f32r, tag="tp")
            for k in range(KC):
                nc.tensor.transpose(
                    pt[:, ds(k * P, P)], h_all[:, ds(t * D + k * P, P)], ident[:]
                )
            ev = nc.scalar.copy if t % 2 == 1 else nc.vector.tensor_copy
            ev(hT3[:, :, ds(t * P, P)],
               pt[:].bitcast(f32).rearrange("p (k x) -> p k x", k=KC))

        transpose_tile(0)
        whu_p = wcast(2, "s0")
        wh_p = wcast(0, "s0")
        transpose_tile(1)
        wu_p = wcast(1, "ua")

        # uw[j,d] = u[j,d]*w_hu[d] + w_h[d];  uwu[j] = sum_d u[j,d]*w_u[d]
        uw = consts.tile([JQ, D], f32r)
        uw0 = consts.tile([JQ, D], f32)
        nc.vector.tensor_mul(uw0[:], u_f, whu_p[:])
        nc.vector.tensor_add(uw[:], uw0[:], wh_p[:])
        uwtmp = consts.tile([JQ, D], f32)
        uwu = consts.tile([JQ, 1], f32)
        nc.vector.scalar_tensor_tensor(
            uwtmp[:], u_f, 1.0, wu_p[:],
            op0=MUL, op1=MUL, accum_out=uwu[:],
        )

        transpose_tile(2)

        # uwT16[d_chunk][j]: 4 transposes into one PSUM bank, one cast-evict
        uwT16 = consts.tile([P, KC * JQ], bf16)
        ptw = ps.tile([P, KC * P], f32r, tag="tp")
        for k in range(KC):
            nc.tensor.transpose(ptw[:, ds(k * P, P)], uw[:, ds(k * P, P)], ident[:])
        nc.scalar.copy(uwT16[:], ptw[:].bitcast(f32))

        transpose_tile(3)

        # ---- scores (transposed layout): sT[j,i] over a 512-wide i-block
        ET16 = consts.tile([JQ, JX], bf16)        # exp(sT + uwu[j]) = exp(s - b)
        m16 = consts.tile([P, NT], f32r)          # per i-tile: max_j ET
        z_rec = consts.tile([P, NT], f32)         # per i-tile: 1/sum_j ET
        mrow = consts.tile([P, 1], f32)

        def block_scores(b):
            sp = ps.tile([JQ, IB], f32, tag="s0")
            for k in range(KC):
                nc.tensor.matmul(
                    sp[:], uwT16[:, ds(k * JQ, JQ)], hT3[:, k, ds(b * IB, IB)],
                    start=(k == 0), stop=(k == KC - 1),
                )
            # ET = exp(sT + uwu[j]); uwu is the per-partition (j) ACT bias
            nc.scalar.activation(ET16[:, ds(b * IB, IB)], sp[:], EXP, bias=uwu[:])

        def block_reduce(b):
            # re-transpose ET (4 tiles into one bank, bf16); batched reduces.
            # For block 1, mrow (the q2c chain head) runs right after MAX.
            et = ps.tile([P, TPB * P], bf16, tag="tp")
            for q in range(TPB):
                t = b * TPB + q
                nc.tensor.transpose(
                    et[:, ds(q * P, P)], ET16[:, ds(t * P, P)], ident16[:]
                )
            et3 = et[:].rearrange("p (q x) -> p q x", q=TPB)
            nc.vector.reduce_max(m16[:, ds(b * TPB, TPB)], et3, axis=AX)
            if b == 1:
                nc.vector.reduce_sum(mrow[:], m16[:].bitcast(f32), axis=AX)
            zsum = stage.tile([P, TPB], f32, tag="zs")
            nc.vector.reduce_sum(zsum[:], et3, axis=AX)
            nc.vector.reciprocal(z_rec[:, ds(b * TPB, TPB)], zsum[:])

        transpose_tile(4)
        block_scores(0)
        transpose_tile(5)
        transpose_tile(6)
        transpose_tile(7)
        block_reduce(0)
        block_scores(1)

        hap = ps.tile([1, D], f32, tag="hap", bufs=1)

        def hap_block(b):
            # q2c accumulation (single PSUM group spanning both blocks)
            for q in range(TPB):
                t = b * TPB + q
                nc.tensor.matmul(
                    hap[:], m16[:, ds(t, 1)], h_all[:, ds(t * D, D)],
                    start=(b == 0 and q == 0), stop=(b == NB - 1 and q == TPB - 1),
                    skip_group_check=True,
                )

        # ---- c2q staging ----
        stgA = consts.tile([P, NT * D], bf16)   # u_a        (out12 col 0)
        stgB = consts.tile([P, NT * D], bf16)   # h * u_a    (out12 col 1)
        ups = {}

        def up_mm(t):
            # ups 4-7 reuse the score/transpose banks (dead by then) so the
            # 2-deep "ua" ring doesn't serialize them behind cols t0-3.
            tag = "ua" if t < 4 else ("s0" if t < 6 else "tp")
            up = ps.tile([P, D], f32, tag=tag)
            ups[t] = up
            nc.tensor.matmul(
                up[:], ET16[:, ds(t * P, P)], u16[:], start=True, stop=True
            )

        def col1(t):
            if t % 2 == 0:
                nc.scalar.mul(stgA[:, ds(t * D, D)], ups[t][:], z_rec[:, ds(t, 1)])
            else:
                nc.vector.tensor_scalar_mul(
                    stgA[:, ds(t * D, D)], ups[t][:], z_rec[:, ds(t, 1)]
                )

        def col2_pair_gps(t0):
            nc.gpsimd.tensor_mul(
                stgB[:, ds(t0 * D, 2 * D)],
                stgA[:, ds(t0 * D, 2 * D)],
                h_f[:, ds(t0 * D, 2 * D)],
            )

        def col2(t):
            nc.vector.scalar_tensor_tensor(
                stgB[:, ds(t * D, D)], ups[t][:], z_rec[:, ds(t, 1)],
                h_f[:, ds(t * D, D)], op0=MUL, op1=MUL,
            )

        def stg_writes(half, nt=TPB):
            # stgA pieces issue on sync, stgB on the scalar HWDGE queue so
            # the tail write issues (~0.65us each) don't serialize.
            for t0 in range(half * TPB, (half + 1) * TPB, nt):
                for col, src, q in ((0, stgA, nc.sync), (1, stgB, nc.scalar)):
                    q.dma_start(
                        out12_d[ds(t0 * P, nt * P), ds(col * D, D)]
                        .rearrange("(t p) c -> p t c", p=P),
                        src[:, ds(t0 * D, nt * D)]
                        .rearrange("p (t c) -> p t c", t=nt),
                    )

        # ---- spine ----
        up_mm(0)
        up_mm(1)
        block_reduce(1)
        up_mm(2)
        up_mm(3)
        hap_block(0)
        hap_block(1)

        # q2c chain: rzq = 1/sum_i m_exp[i] folds into the hap eviction
        # (ha_row = normalized h_a); haT = per-chunk column form of h_a so
        # o4T = hT16 * haT[k] is a per-partition tensor_scalar in bf16.
        zqp = ps.tile([1, 1], f32, tag="acc", bufs=1)
        nc.tensor.matmul(zqp[:], mrow[:], ones_col[:], start=True, stop=True)
        rzq = consts.tile([1, 1], f32)
        nc.vector.reciprocal(rzq[:], zqp[:])
        up_mm(4)
        up_mm(5)
        up_mm(6)
        up_mm(7)
        # ha_row is the UNNORMALIZED hap evict (no rzq wait); 1/zq is
        # partition-broadcast by a K=1 matmul into a dead tp bank and folded
        # into the tiny haT evicts instead.
        ha_row = consts.tile([1, D], bf16)
        nc.scalar.copy(ha_row[:], hap[:])
        rzq_bc = ps.tile([P, 1], f32, tag="hap", bufs=1)
        nc.tensor.matmul(rzq_bc[:], ones_row_f[:], rzq[:], start=True, stop=True)
        # bf16 PSUM writes must stay 4-byte aligned -> pad each haT column
        # to a 2-element stride and gather on the evict.
        haT_p = ps.tile([P, 2 * KC], bf16, tag="acc", bufs=1)
        for k in range(KC):
            nc.tensor.transpose(
                haT_p[:, ds(2 * k, 1)], ha_row[:, ds(k * P, P)],
                ident16[ds(0, 1), ds(0, 1)],
            )
        haT_strided = haT_p[:].rearrange("p (k two) -> p k two", two=2)[:, :, 0]
        haT = consts.tile([P, KC], f32)
        nc.vector.tensor_scalar_mul(haT[:], haT_strided, rzq_bc[:])
        haT16 = consts.tile([P, KC], bf16)
        nc.vector.tensor_scalar_mul(haT16[:], haT_strided, rzq_bc[:])

        o4T16 = consts.tile([P, KC * JX], bf16)

        def o4T_chunk(k, eng):
            if eng is nc.scalar:
                # InstActivation Copy+scale: per-partition scale, fast path
                nc.scalar.mul(o4T16[:, ds(k * JX, JX)], hT3[:, k, :],
                              haT[:, ds(k, 1)])
            else:
                # all-bf16 TT with a stride-0 broadcast scalar: 2x fast path
                # (TensorScalarPtr with bf16 SBUF in0 is a ~15us ucode path)
                eng.tensor_mul(
                    o4T16[:, ds(k * JX, JX)].rearrange("p (o x) -> p o x", o=1),
                    hT3[:, k, :].rearrange("p (o x) -> p o x", o=1),
                    haT16[:, ds(k, 1)].rearrange("p (o c) -> p o c", o=1)
                    .broadcast_to([P, 1, JX]),
                )

        def o4T_write(k0, nk=2):
            for k in range(k0, k0 + nk):
                q = nc.sync if k % 2 == 0 else nc.scalar
                q.dma_start(
                    o4T_d[ds(k * P, P), :],
                    o4T16[:, ds(k * JX, JX)],
                )

        col1(0)
        col1(1)
        col2_pair_gps(0)
        col1(2)
        col1(3)
        col2(2)
        col2(3)
        o4T_chunk(0, nc.scalar)
        o4T_chunk(1, nc.gpsimd)
        col1(4)
        col1(5)
        col2(4)
        col2(5)
        stg_writes(0)
        o4T_chunk(2, nc.scalar)
        o4T_write(0)
        col1(6)
        col1(7)
        o4T_chunk(3, nc.gpsimd)
        col2(6)
        col2(7)
        o4T_write(2, nk=1)
        stg_writes(1, nt=2)
        o4T_write(3, nk=1)

    nc.compile()
    return nc


def _get_nc():
    if "nc" not in _CACHE:
        _CACHE["nc"] = _build_program()
    return _CACHE["nc"]


def _ensure_axon_hooks_stub():
    # concourse imports antenv.axon_hooks when tracing is requested via env;
    # provide a no-op stub if the image lacks it so runs degrade gracefully.
    import sys
    import types

    try:
        import antenv.axon_hooks  # noqa: F401
    except ImportError:
        mod = types.ModuleType("antenv.axon_hooks")
        _hook = [None]
        mod.set_axon_ntff_profile_hook = lambda hook: _hook.__setitem__(0, hook)
        mod.get_axon_ntff_profile_hook = lambda: _hook[0]
        sys.modules["antenv.axon_hooks"] = mod


def _postprocess(res):
    out = np.empty((N_B, JX, 4 * D), dtype=np.float32)
    for n in range(N_B):
        r = res.results[n]
        out[n, :, :D] = np.asarray(r["out0"])
        out[n, :, D:3 * D] = np.asarray(r["out12"]).astype(np.float32)
        out[n, :, 3 * D:] = np.asarray(r["o4T"]).astype(np.float32).T
    return out.reshape(N_B, M_B, JX, 4 * D)


def kernel(h, u, alpha_w, alpha_b=None, **_unused):
    _ensure_axon_hooks_stub()
    from concourse.bass_utils import run_bass_kernel_spmd

    h = np.ascontiguousarray(np.asarray(h, dtype=np.float32)).reshape(N_B, JX, D)
    u = np.ascontiguousarray(np.asarray(u, dtype=np.float32)).reshape(N_B, JQ, D)
    alpha_w = np.ascontiguousarray(np.asarray(alpha_w, dtype=np.float32)).reshape(3 * D)

    nc = _get_nc()
    in_maps = [
        {"h": h[n], "u": u[n], "alpha_w": alpha_w} for n in range(N_B)
    ]
    res = run_bass_kernel_spmd(nc, in_maps, core_ids=list(range(N_B)))
    return _postprocess(res)


# revision 28
# speedup vs baseline: 1.0622x; 1.0622x over previous
"""Trainium2 Bass kernel for BiDAF-style bidirectional attention.

Reference computation (per batch element n; M=1 folded away):
    s[i,j]  = h[i].w_h + u[j].w_u + (h[i]*u[j]).w_hu + b      [JX, JQ]
    a_u     = softmax_j(s);     u_a[i] = sum_j a_u[i,j] u[j]   (c2q)
    a_h     = softmax_i(max_j s);  h_a = sum_i a_h[i] h[i]     (q2c)
    out     = concat(h, u_a, h*u_a, h*h_a)                     [JX, 4D]

Sharding: data-parallel over batch N=8, one NeuronCore per batch element.
alpha_b drops out entirely (both softmaxes are shift-invariant).

Final structure (51us f32 baseline -> ~40us):
  - outputs split by layout: out0 f32 [JX, D] = h (one DRAM->DRAM DMA, gated
    to run after the loads: its big descriptors win the SDMA per-packet
    round-robin and would starve them); out12 bf16 [JX, 2D] = u_a | h*u_a;
    o4T bf16 [D, JX] = (h*h_a) TRANSPOSED.  The host stitches/transposes.
    bf16 stores halve traffic; the 2e-2 rel-err gate leaves ~5x margin.
  - o4T trick: h_a is per-PARTITION in the hT layout, so h*h_a is four
    all-bf16 tensor_scalar ops over [128, JX] with no PSUM broadcast, no
    bc matmul, and no SBUF broadcast copy.
  - bf16 matmul operands (hT, ET, u, uw'); f32 accumulation in PSUM.
  - few big DMAs; h loads as 8 singles so completions stagger and the
    transpose pipeline chases them.
  - PE spine: t0..t3+uwT transposes -> scores-b0 -> t4..t7 -> etr-b0 ->
    scores-b1 -> c2q ups interleaved with etr-b1/hap -> q2c chain.
  - elementwise tail: col1 = up*zr (ACT, PSUM-read), col2 = (up*zr)*h as
    f32-in DVE stt (mixed bf16xf32 tensor_tensor on DVE is a 4x-slow
    path - avoided), o4T on DVE/GpSimd.
"""

import numpy as np

N_B, M_B, JX, JQ, D = 8, 1, 1024, 128, 512
P = 128
NT = JX // P   # 8 i-tiles
KC = D // P    # 4 d-chunks
IB = 512       # i-block width for score matmuls
NB = JX // IB  # 2 blocks
TPB = NT // NB  # tiles per block

_CACHE = {}


def _build_program():
    from contextlib import ExitStack

    import concourse.bass as bass
    import concourse.tile as tile
    from concourse import bacc, mybir
    from concourse.masks import make_identity
    from concourse.tile_rust import add_dep_helper

    f32 = mybir.dt.float32
    f32r = mybir.dt.float32r
    bf16 = mybir.dt.bfloat16
    EXP = mybir.ActivationFunctionType.Exp
    AX = mybir.AxisListType.X
    MUL = mybir.AluOpType.mult
    ds = bass.ds

    nc = bacc.Bacc("TRN2", target_bir_lowering=False, debug=False, num_devices=8)
    h_d = nc.dram_tensor("h", [JX, D], f32, kind="ExternalInput").ap()
    u_d = nc.dram_tensor("u", [JQ, D], f32, kind="ExternalInput").ap()
    aw_d = nc.dram_tensor("alpha_w", [3 * D], f32, kind="ExternalInput").ap()
    out0_d = nc.dram_tensor("out0", [JX, D], f32, kind="ExternalOutput").ap()
    out12_d = nc.dram_tensor("out12", [JX, 2 * D], bf16, kind="ExternalOutput").ap()
    o4T_d = nc.dram_tensor("o4T", [D, JX], bf16, kind="ExternalOutput").ap()

    with tile.TileContext(nc) as tc, ExitStack() as ctx:
        consts = ctx.enter_context(tc.tile_pool(name="consts", bufs=1))
        stage = ctx.enter_context(tc.tile_pool(name="stage", bufs=4))
        # PSUM budget (8 banks): tp=2, s0=2, ua=2, acc=1, hap=1
        ps = ctx.enter_context(tc.tile_pool(name="ps", bufs=2, space="PSUM"))

        # ---- PE warmup: opens the HAM clock-gate (1.2 -> 2.4 GHz) while
        # the first h loads stream in.
        warm_f = consts.tile([P, D], f32)
        nc.vector.memset(warm_f[:], 0.25)
        warm = consts.tile([P, D], f32r)
        nc.vector.tensor_copy(warm[:], warm_f[:])
        wp = ps.tile([P, D], f32, tag="acc", bufs=1)
        for w in range(2):
            nc.tensor.matmul(
                wp[:], warm[:, ds(0, P)], warm[:], start=True, stop=True,
            )

        # ---- constants ----
        ident_f = consts.tile([P, P], f32)
        make_identity(nc, ident_f[:])
        ident = consts.tile([P, P], f32r)
        nc.vector.tensor_copy(ident[:], ident_f[:])
        ident16 = consts.tile([P, P], bf16)
        nc.vector.tensor_copy(ident16[:], ident_f[:])
        ones_row_f = consts.tile([1, P], f32)
        nc.vector.memset(ones_row_f[:], 1.0)
        ones_row = consts.tile([1, P], f32r)
        nc.scalar.copy(ones_row[:], ones_row_f[:])
        ones_col = consts.tile([P, 1], f32)
        nc.vector.memset(ones_col[:], 1.0)

        # ---- loads (sync queue): h0, h1 first (they gate the transpose
        # pipeline), then aw + u (uw' prep chain), then h2-h7.
        h_all = consts.tile([P, NT * D], f32r)    # tile t: h[t*128+p, d]
        h_f = h_all[:].bitcast(f32)
        for t in range(2):
            nc.sync.dma_start(
                h_all[:, ds(t * D, D)], h_d[ds(t * P, P), :].bitcast(f32r)
            )
        aw_sb = consts.tile([1, 3 * D], f32r)
        nc.sync.dma_start(aw_sb[:], aw_d.rearrange("(o d) -> o d", o=1).bitcast(f32r))
        u_sb = consts.tile([JQ, D], f32r)
        nc.sync.dma_start(u_sb[:], u_d[:].bitcast(f32r))
        u_f = u_sb[:].bitcast(f32)
        last_h = None
        for t in range(2, NT):
            last_h = nc.sync.dma_start(
                h_all[:, ds(t * D, D)], h_d[ds(t * P, P), :].bitcast(f32r)
            )

        # passthrough column: DRAM->DRAM copy of h in 8KB rows, gated out of
        # the load window (see docstring).
        d2d = nc.sync.dma_start(
            out0_d.rearrange("(a b) d -> a (b d)", b=4),
            h_d.rearrange("(a b) d -> a (b d)", b=4),
        )
        add_dep_helper(d2d.ins, last_h.ins, sync=True,
                       reason="keep d2d pass copy out of the load window")

        u16 = consts.tile([JQ, D], bf16)
        nc.gpsimd.tensor_copy(u16[:], u_f)

        # alpha_w partition-broadcast on-chip: K=1 matmuls into PSUM, read
        # directly by the DVE.  whu first (gates uw').  wu goes to the "ua"
        # slot so it doesn't wait for whu's bank to free (that wait jammed
        # the PE FIFO ahead of the t2/t3 transposes).
        def wcast(c, tag):
            wt = ps.tile([P, D], f32, tag=tag)
            nc.tensor.matmul(
                wt[:], ones_row[:], aw_sb[:, ds(c * D, D)], start=True, stop=True
            )
            return wt

        # ---- hT via PE transposes (f32r ident moving), cast-evict to bf16.
        hT16 = consts.tile([P, KC * JX], bf16)    # chunk k: hT[k*128+p, i]
        hT3 = hT16[:].rearrange("p (k x) -> p k x", k=KC)

        def transpose_tile(t):
            pt = ps.tile([P, KC * P], f32r, tag="tp")
            for k in range(KC):
                nc.tensor.transpose(
                    pt[:, ds(k * P, P)], h_all[:, ds(t * D + k * P, P)], ident[:]
                )
            ev = nc.scalar.copy if t in (1, 3) else nc.vector.tensor_copy
            ev(hT3[:, :, ds(t * P, P)],
               pt[:].bitcast(f32).rearrange("p (k x) -> p k x", k=KC))

        transpose_tile(0)
        whu_p = wcast(2, "s0")
        wh_p = wcast(0, "s0")
        transpose_tile(1)
        wu_p = wcast(1, "ua")

        # uw[j,d] = u[j,d]*w_hu[d] + w_h[d];  uwu[j] = sum_d u[j,d]*w_u[d]
        uw = consts.tile([JQ, D], f32r)
        uw0 = consts.tile([JQ, D], f32)
        nc.vector.tensor_mul(uw0[:], u_f, whu_p[:])
        nc.vector.tensor_add(uw[:], uw0[:], wh_p[:])
        uwtmp = consts.tile([JQ, D], f32)
        uwu = consts.tile([JQ, 1], f32)
        nc.vector.scalar_tensor_tensor(
            uwtmp[:], u_f, 1.0, wu_p[:],
            op0=MUL, op1=MUL, accum_out=uwu[:],
        )

        transpose_tile(2)

        # uwT16[d_chunk][j]: 4 transposes into one PSUM bank, one cast-evict
        uwT16 = consts.tile([P, KC * JQ], bf16)
        ptw = ps.tile([P, KC * P], f32r, tag="tp")
        for k in range(KC):
            nc.tensor.transpose(ptw[:, ds(k * P, P)], uw[:, ds(k * P, P)], ident[:])
        nc.scalar.copy(uwT16[:], ptw[:].bitcast(f32))

        transpose_tile(3)

        # ---- scores (transposed layout): sT[j,i] over a 512-wide i-block
        ET16 = consts.tile([JQ, JX], bf16)        # exp(sT + uwu[j]) = exp(s - b)
        m16 = consts.tile([P, NT], f32r)          # per i-tile: max_j ET
        z_rec = consts.tile([P, NT], f32)         # per i-tile: 1/sum_j ET
        mrow = consts.tile([P, 1], f32)

        def block_scores(b):
            sp = ps.tile([JQ, IB], f32, tag="s0")
            for k in range(KC):
                nc.tensor.matmul(
                    sp[:], uwT16[:, ds(k * JQ, JQ)], hT3[:, k, ds(b * IB, IB)],
                    start=(k == 0), stop=(k == KC - 1),
                )
            # ET = exp(sT + uwu[j]); uwu is the per-partition (j) ACT bias
            nc.scalar.activation(ET16[:, ds(b * IB, IB)], sp[:], EXP, bias=uwu[:])

        def block_reduce(b):
            # re-transpose ET (4 tiles into one bank, bf16); batched reduces.
            # For block 1, mrow (the q2c chain head) runs right after MAX.
            et = ps.tile([P, TPB * P], bf16, tag="tp")
            for q in range(TPB):
                t = b * TPB + q
                nc.tensor.transpose(
                    et[:, ds(q * P, P)], ET16[:, ds(t * P, P)], ident16[:]
                )
            et3 = et[:].rearrange("p (q x) -> p q x", q=TPB)
            nc.vector.reduce_max(m16[:, ds(b * TPB, TPB)], et3, axis=AX)
            if b == 1:
                nc.vector.reduce_sum(mrow[:], m16[:].bitcast(f32), axis=AX)
            zsum = stage.tile([P, TPB], f32, tag="zs")
            nc.vector.reduce_sum(zsum[:], et3, axis=AX)
            nc.vector.reciprocal(z_rec[:, ds(b * TPB, TPB)], zsum[:])

        transpose_tile(4)
        block_scores(0)
        transpose_tile(5)
        transpose_tile(6)
        transpose_tile(7)
        block_reduce(0)
        block_scores(1)

        hap = ps.tile([1, D], f32, tag="hap", bufs=1)

        def hap_block(b):
            # q2c accumulation (single PSUM group spanning both blocks)
            for q in range(TPB):
                t = b * TPB + q
                nc.tensor.matmul(
                    hap[:], m16[:, ds(t, 1)], h_all[:, ds(t * D, D)],
                    start=(b == 0 and q == 0), stop=(b == NB - 1 and q == TPB - 1),
                    skip_group_check=True,
                )

        # ---- c2q staging ----
        stgA = consts.tile([P, NT * D], bf16)   # u_a        (out12 col 0)
        stgB = consts.tile([P, NT * D], bf16)   # h * u_a    (out12 col 1)
        ups = {}

        def up_mm(t):
            # ups 4-7 reuse the score/transpose banks (dead by then) so the
            # 2-deep "ua" ring doesn't serialize them behind cols t0-3.
            tag = "ua" if t < 4 else ("s0" if t < 6 else "tp")
            up = ps.tile([P, D], f32, tag=tag)
            ups[t] = up
            nc.tensor.matmul(
                up[:], ET16[:, ds(t * P, P)], u16[:], start=True, stop=True
            )

        def col1(t):
            if t % 2 == 0:
                nc.scalar.mul(stgA[:, ds(t * D, D)], ups[t][:], z_rec[:, ds(t, 1)])
            else:
                nc.vector.tensor_scalar_mul(
                    stgA[:, ds(t * D, D)], ups[t][:], z_rec[:, ds(t, 1)]
                )

        def col2_pair_gps(t0):
            nc.gpsimd.tensor_mul(
                stgB[:, ds(t0 * D, 2 * D)],
                stgA[:, ds(t0 * D, 2 * D)],
                h_f[:, ds(t0 * D, 2 * D)],
            )

        def col2(t):
            nc.vector.scalar_tensor_tensor(
                stgB[:, ds(t * D, D)], ups[t][:], z_rec[:, ds(t, 1)],
                h_f[:, ds(t * D, D)], op0=MUL, op1=MUL,
            )

        def stg_writes(half, nt=TPB):
            # stgA pieces issue on sync, stgB on the scalar HWDGE queue so
            # the tail write issues (~0.65us each) don't serialize.
            for t0 in range(half * TPB, (half + 1) * TPB, nt):
                for col, src, q in ((0, stgA, nc.sync), (1, stgB, nc.scalar)):
                    q.dma_start(
                        out12_d[ds(t0 * P, nt * P), ds(col * D, D)]
                        .rearrange("(t p) c -> p t c", p=P),
                        src[:, ds(t0 * D, nt * D)]
                        .rearrange("p (t c) -> p t c", t=nt),
                    )

        # ---- spine ----
        up_mm(0)
        up_mm(1)
        block_reduce(1)
        up_mm(2)
        up_mm(3)
        hap_block(0)
        hap_block(1)

        # q2c chain: rzq = 1/sum_i m_exp[i] folds into the hap eviction
        # (ha_row = normalized h_a); haT = per-chunk column form of h_a so
        # o4T = hT16 * haT[k] is a per-partition tensor_scalar in bf16.
        zqp = ps.tile([1, 1], f32, tag="acc", bufs=1)
        nc.tensor.matmul(zqp[:], mrow[:], ones_col[:], start=True, stop=True)
        rzq = consts.tile([1, 1], f32)
        nc.vector.reciprocal(rzq[:], zqp[:])
        up_mm(4)
        up_mm(5)
        up_mm(6)
        up_mm(7)
        # ha_row is the UNNORMALIZED hap evict (no rzq wait); 1/zq is
        # partition-broadcast by a K=1 matmul into a dead tp bank and folded
        # into the tiny haT evicts instead.
        ha_row = consts.tile([1, D], bf16)
        nc.scalar.copy(ha_row[:], hap[:])
        rzq_bc = ps.tile([P, 1], f32, tag="hap", bufs=1)
        nc.tensor.matmul(rzq_bc[:], ones_row_f[:], rzq[:], start=True, stop=True)
        # bf16 PSUM writes must stay 4-byte aligned -> pad each haT column
        # to a 2-element stride and gather on the evict.
        haT_p = ps.tile([P, 2 * KC], bf16, tag="acc", bufs=1)
        for k in range(KC):
            nc.tensor.transpose(
                haT_p[:, ds(2 * k, 1)], ha_row[:, ds(k * P, P)],
                ident16[ds(0, 1), ds(0, 1)],
            )
        haT_strided = haT_p[:].rearrange("p (k two) -> p k two", two=2)[:, :, 0]
        haT = consts.tile([P, KC], f32)
        nc.vector.tensor_scalar_mul(haT[:], haT_strided, rzq_bc[:])
        haT16 = consts.tile([P, KC], bf16)
        nc.vector.tensor_scalar_mul(haT16[:], haT_strided, rzq_bc[:])

        o4T16 = consts.tile([P, KC * JX], bf16)

        def o4T_chunk(k, eng):
            if eng is nc.scalar:
                # InstActivation Copy+scale: per-partition scale, fast path
                nc.scalar.mul(o4T16[:, ds(k * JX, JX)], hT3[:, k, :],
                              haT[:, ds(k, 1)])
            else:
                # all-bf16 TT with a stride-0 broadcast scalar: 2x fast path
                # (TensorScalarPtr with bf16 SBUF in0 is a ~15us ucode path)
                eng.tensor_mul(
                    o4T16[:, ds(k * JX, JX)].rearrange("p (o x) -> p o x", o=1),
                    hT3[:, k, :].rearrange("p (o x) -> p o x", o=1),
                    haT16[:, ds(k, 1)].rearrange("p (o c) -> p o c", o=1)
                    .broadcast_to([P, 1, JX]),
                )

        def o4T_write(k0, nk=2):
            for k in range(k0, k0 + nk):
                q = nc.sync if k % 2 == 0 else nc.scalar
                q.dma_start(
                    o4T_d[ds(k * P, P), :],
                    o4T16[:, ds(k * JX, JX)],
                )

        col1(0)
        col1(1)
        col2_pair_gps(0)
        col1(2)
        col1(3)
        col2(2)
        col2(3)
        o4T_chunk(0, nc.scalar)
        o4T_chunk(1, nc.gpsimd)
        col1(4)
        col1(5)
        col2(4)
        col2(5)
        stg_writes(0)
        o4T_chunk(2, nc.scalar)
        o4T_write(0)
        col1(6)
        col1(7)
        o4T_chunk(3, nc.gpsimd)
        col2(6)
        col2(7)
        o4T_write(2, nk=1)
        stg_writes(1, nt=2)
        o4T_write(3, nk=1)

    nc.compile()
    return nc


def _get_nc():
    if "nc" not in _CACHE:
        _CACHE["nc"] = _build_program()
    return _CACHE["nc"]


def _ensure_axon_hooks_stub():
    # concourse imports antenv.axon_hooks when tracing is requested via env;
    # provide a no-op stub if the image lacks it so runs degrade gracefully.
    import sys
    import types

    try:
        import antenv.axon_hooks  # noqa: F401
    except ImportError:
        mod = types.ModuleType("antenv.axon_hooks")
        _hook = [None]
        mod.set_axon_ntff_profile_hook = lambda hook: _hook.__setitem__(0, hook)
        mod.get_axon_ntff_profile_hook = lambda: _hook[0]
        sys.modules["antenv.axon_hooks"] = mod


def _postprocess(res):
    out = np.empty((N_B, JX, 4 * D), dtype=np.float32)
    for n in range(N_B):
        r = res.results[n]
        out[n, :, :D] = np.asarray(r["out0"])
        out[n, :, D:3 * D] = np.asarray(r["out12"]).astype(np.float32)
        out[n, :, 3 * D:] = np.asarray(r["o4T"]).astype(np.float32).T
    return out.reshape(N_B, M_B, JX, 4 * D)


def kernel(h, u, alpha_w, alpha_b=None, **_unused):
    _ensure_axon_hooks_stub()
    from concourse.bass_utils import run_bass_kernel_spmd

    h = np.ascontiguousarray(np.asarray(h, dtype=np.float32)).reshape(N_B, JX, D)
    u = np.ascontiguousarray(np.asarray(u, dtype=np.float32)).reshape(N_B, JQ, D)
    alpha_w = np.ascontiguousarray(np.asarray(alpha_w, dtype=np.float32)).reshape(3 * D)

    nc = _get_nc()
    in_maps = [
        {"h": h[n], "u": u[n], "alpha_w": alpha_w} for n in range(N_B)
    ]
    res = run_bass_kernel_spmd(nc, in_maps, core_ids=list(range(N_B)))
    return _postprocess(res)


# revision 29
# speedup vs baseline: 1.0644x; 1.0021x over previous
"""Trainium2 Bass kernel for BiDAF-style bidirectional attention.

Reference computation (per batch element n; M=1 folded away):
    s[i,j]  = h[i].w_h + u[j].w_u + (h[i]*u[j]).w_hu + b      [JX, JQ]
    a_u     = softmax_j(s);     u_a[i] = sum_j a_u[i,j] u[j]   (c2q)
    a_h     = softmax_i(max_j s);  h_a = sum_i a_h[i] h[i]     (q2c)
    out     = concat(h, u_a, h*u_a, h*h_a)                     [JX, 4D]

Sharding: data-parallel over batch N=8, one NeuronCore per batch element.
alpha_b drops out entirely (both softmaxes are shift-invariant).

Final structure (51us f32 baseline -> ~40us):
  - outputs split by layout: out0 f32 [JX, D] = h (one DRAM->DRAM DMA, gated
    to run after the loads: its big descriptors win the SDMA per-packet
    round-robin and would starve them); out12 bf16 [JX, 2D] = u_a | h*u_a;
    o4T bf16 [D, JX] = (h*h_a) TRANSPOSED.  The host stitches/transposes.
    bf16 stores halve traffic; the 2e-2 rel-err gate leaves ~5x margin.
  - o4T trick: h_a is per-PARTITION in the hT layout, so h*h_a is four
    all-bf16 tensor_scalar ops over [128, JX] with no PSUM broadcast, no
    bc matmul, and no SBUF broadcast copy.
  - bf16 matmul operands (hT, ET, u, uw'); f32 accumulation in PSUM.
  - few big DMAs; h loads as 8 singles so completions stagger and the
    transpose pipeline chases them.
  - PE spine: t0..t3+uwT transposes -> scores-b0 -> t4..t7 -> etr-b0 ->
    scores-b1 -> c2q ups interleaved with etr-b1/hap -> q2c chain.
  - elementwise tail: col1 = up*zr (ACT, PSUM-read), col2 = (up*zr)*h as
    f32-in DVE stt (mixed bf16xf32 tensor_tensor on DVE is a 4x-slow
    path - avoided), o4T on DVE/GpSimd.
"""

import numpy as np

N_B, M_B, JX, JQ, D = 8, 1, 1024, 128, 512
P = 128
NT = JX // P   # 8 i-tiles
KC = D // P    # 4 d-chunks
IB = 512       # i-block width for score matmuls
NB = JX // IB  # 2 blocks
TPB = NT // NB  # tiles per block

_CACHE = {}


def _build_program():
    from contextlib import ExitStack

    import concourse.bass as bass
    import concourse.tile as tile
    from concourse import bacc, mybir
    from concourse.masks import make_identity
    from concourse.tile_rust import add_dep_helper

    f32 = mybir.dt.float32
    f32r = mybir.dt.float32r
    bf16 = mybir.dt.bfloat16
    EXP = mybir.ActivationFunctionType.Exp
    AX = mybir.AxisListType.X
    MUL = mybir.AluOpType.mult
    ds = bass.ds

    nc = bacc.Bacc("TRN2", target_bir_lowering=False, debug=False, num_devices=8)
    h_d = nc.dram_tensor("h", [JX, D], f32, kind="ExternalInput").ap()
    u_d = nc.dram_tensor("u", [JQ, D], f32, kind="ExternalInput").ap()
    aw_d = nc.dram_tensor("alpha_w", [3 * D], f32, kind="ExternalInput").ap()
    out0_d = nc.dram_tensor("out0", [JX, D], f32, kind="ExternalOutput").ap()
    out12_d = nc.dram_tensor("out12", [JX, 2 * D], bf16, kind="ExternalOutput").ap()
    o4T_d = nc.dram_tensor("o4T", [D, JX], bf16, kind="ExternalOutput").ap()

    with tile.TileContext(nc) as tc, ExitStack() as ctx:
        consts = ctx.enter_context(tc.tile_pool(name="consts", bufs=1))
        stage = ctx.enter_context(tc.tile_pool(name="stage", bufs=4))
        # PSUM budget (8 banks): tp=2, s0=2, ua=2, acc=1, hap=1
        ps = ctx.enter_context(tc.tile_pool(name="ps", bufs=2, space="PSUM"))

        # ---- PE warmup: opens the HAM clock-gate (1.2 -> 2.4 GHz) while
        # the first h loads stream in.
        warm_f = consts.tile([P, D], f32)
        nc.vector.memset(warm_f[:], 0.25)
        warm = consts.tile([P, D], f32r)
        nc.vector.tensor_copy(warm[:], warm_f[:])
        wp = ps.tile([P, D], f32, tag="acc", bufs=1)
        for w in range(2):
            nc.tensor.matmul(
                wp[:], warm[:, ds(0, P)], warm[:], start=True, stop=True,
            )

        # ---- constants ----
        ident_f = consts.tile([P, P], f32)
        make_identity(nc, ident_f[:])
        ident = consts.tile([P, P], f32r)
        nc.vector.tensor_copy(ident[:], ident_f[:])
        ident16 = consts.tile([P, P], bf16)
        nc.vector.tensor_copy(ident16[:], ident_f[:])
        ones_row_f = consts.tile([1, P], f32)
        nc.vector.memset(ones_row_f[:], 1.0)
        ones_row = consts.tile([1, P], f32r)
        nc.scalar.copy(ones_row[:], ones_row_f[:])
        ones_col = consts.tile([P, 1], f32)
        nc.vector.memset(ones_col[:], 1.0)

        # ---- loads (sync queue): h0, h1 first (they gate the transpose
        # pipeline), then aw + u (uw' prep chain), then h2-h7.
        h_all = consts.tile([P, NT * D], f32r)    # tile t: h[t*128+p, d]
        h_f = h_all[:].bitcast(f32)
        for t in range(2):
            nc.sync.dma_start(
                h_all[:, ds(t * D, D)], h_d[ds(t * P, P), :].bitcast(f32r)
            )
        aw_sb = consts.tile([1, 3 * D], f32r)
        nc.sync.dma_start(aw_sb[:], aw_d.rearrange("(o d) -> o d", o=1).bitcast(f32r))
        u_sb = consts.tile([JQ, D], f32r)
        nc.sync.dma_start(u_sb[:], u_d[:].bitcast(f32r))
        u_f = u_sb[:].bitcast(f32)
        last_h = None
        for t in range(2, NT):
            last_h = nc.sync.dma_start(
                h_all[:, ds(t * D, D)], h_d[ds(t * P, P), :].bitcast(f32r)
            )

        # passthrough column: DRAM->DRAM copy of h in 8KB rows, gated out of
        # the load window (see docstring).
        d2d = nc.sync.dma_start(
            out0_d.rearrange("(a b) d -> a (b d)", b=4),
            h_d.rearrange("(a b) d -> a (b d)", b=4),
        )
        add_dep_helper(d2d.ins, last_h.ins, sync=True,
                       reason="keep d2d pass copy out of the load window")

        u16 = consts.tile([JQ, D], bf16)
        nc.gpsimd.tensor_copy(u16[:], u_f)

        # alpha_w partition-broadcast on-chip: K=1 matmuls into PSUM, read
        # directly by the DVE.  whu first (gates uw').  wu goes to the "ua"
        # slot so it doesn't wait for whu's bank to free (that wait jammed
        # the PE FIFO ahead of the t2/t3 transposes).
        def wcast(c, tag):
            wt = ps.tile([P, D], f32, tag=tag)
            nc.tensor.matmul(
                wt[:], ones_row[:], aw_sb[:, ds(c * D, D)], start=True, stop=True
            )
            return wt

        # ---- hT via PE transposes (f32r ident moving), cast-evict to bf16.
        hT16 = consts.tile([P, KC * JX], bf16)    # chunk k: hT[k*128+p, i]
        hT3 = hT16[:].rearrange("p (k x) -> p k x", k=KC)

        def transpose_tile(t):
            # t4/t6 borrow the acc bank (dead between warmup and the q2c
            # chain) as a third transpose buffer so the tp ring's
            # transpose->evict lockstep doesn't throttle the t5..t7 path.
            tag = "acc" if t in (4, 6) else "tp"
            bufs = 1 if t in (4, 6) else 2
            pt = ps.tile([P, KC * P], f32r, tag=tag, bufs=bufs)
            for k in range(KC):
                nc.tensor.transpose(
                    pt[:, ds(k * P, P)], h_all[:, ds(t * D + k * P, P)], ident[:]
                )
            ev = nc.scalar.copy if t in (1, 3) else nc.vector.tensor_copy
            ev(hT3[:, :, ds(t * P, P)],
               pt[:].bitcast(f32).rearrange("p (k x) -> p k x", k=KC))

        transpose_tile(0)
        whu_p = wcast(2, "s0")
        wh_p = wcast(0, "s0")
        transpose_tile(1)
        wu_p = wcast(1, "ua")

        # uw[j,d] = u[j,d]*w_hu[d] + w_h[d];  uwu[j] = sum_d u[j,d]*w_u[d]
        uw = consts.tile([JQ, D], f32r)
        uw0 = consts.tile([JQ, D], f32)
        nc.vector.tensor_mul(uw0[:], u_f, whu_p[:])
        nc.vector.tensor_add(uw[:], uw0[:], wh_p[:])
        uwtmp = consts.tile([JQ, D], f32)
        uwu = consts.tile([JQ, 1], f32)
        nc.vector.scalar_tensor_tensor(
            uwtmp[:], u_f, 1.0, wu_p[:],
            op0=MUL, op1=MUL, accum_out=uwu[:],
        )

        transpose_tile(2)

        # uwT16[d_chunk][j]: 4 transposes into one PSUM bank, one cast-evict
        uwT16 = consts.tile([P, KC * JQ], bf16)
        ptw = ps.tile([P, KC * P], f32r, tag="tp")
        for k in range(KC):
            nc.tensor.transpose(ptw[:, ds(k * P, P)], uw[:, ds(k * P, P)], ident[:])
        nc.scalar.copy(uwT16[:], ptw[:].bitcast(f32))

        transpose_tile(3)

        # ---- scores (transposed layout): sT[j,i] over a 512-wide i-block
        ET16 = consts.tile([JQ, JX], bf16)        # exp(sT + uwu[j]) = exp(s - b)
        m16 = consts.tile([P, NT], f32r)          # per i-tile: max_j ET
        z_rec = consts.tile([P, NT], f32)         # per i-tile: 1/sum_j ET
        mrow = consts.tile([P, 1], f32)

        def block_scores(b):
            sp = ps.tile([JQ, IB], f32, tag="s0")
            for k in range(KC):
                nc.tensor.matmul(
                    sp[:], uwT16[:, ds(k * JQ, JQ)], hT3[:, k, ds(b * IB, IB)],
                    start=(k == 0), stop=(k == KC - 1),
                )
            # ET = exp(sT + uwu[j]); uwu is the per-partition (j) ACT bias
            nc.scalar.activation(ET16[:, ds(b * IB, IB)], sp[:], EXP, bias=uwu[:])

        def block_reduce(b):
            # re-transpose ET (4 tiles into one bank, bf16); batched reduces.
            # For block 1, mrow (the q2c chain head) runs right after MAX.
            et = ps.tile([P, TPB * P], bf16, tag="tp")
            for q in range(TPB):
                t = b * TPB + q
                nc.tensor.transpose(
                    et[:, ds(q * P, P)], ET16[:, ds(t * P, P)], ident16[:]
                )
            et3 = et[:].rearrange("p (q x) -> p q x", q=TPB)
            nc.vector.reduce_max(m16[:, ds(b * TPB, TPB)], et3, axis=AX)
            if b == 1:
                nc.vector.reduce_sum(mrow[:], m16[:].bitcast(f32), axis=AX)
            zsum = stage.tile([P, TPB], f32, tag="zs")
            nc.vector.reduce_sum(zsum[:], et3, axis=AX)
            nc.vector.reciprocal(z_rec[:, ds(b * TPB, TPB)], zsum[:])

        transpose_tile(4)
        block_scores(0)
        transpose_tile(5)
        transpose_tile(6)
        transpose_tile(7)
        block_reduce(0)
        block_scores(1)

        hap = ps.tile([1, D], f32, tag="hap", bufs=1)

        def hap_block(b):
            # q2c accumulation (single PSUM group spanning both blocks)
            for q in range(TPB):
                t = b * TPB + q
                nc.tensor.matmul(
                    hap[:], m16[:, ds(t, 1)], h_all[:, ds(t * D, D)],
                    start=(b == 0 and q == 0), stop=(b == NB - 1 and q == TPB - 1),
                    skip_group_check=True,
                )

        # ---- c2q staging ----
        stgA = consts.tile([P, NT * D], bf16)   # u_a        (out12 col 0)
        stgB = consts.tile([P, NT * D], bf16)   # h * u_a    (out12 col 1)
        ups = {}

        def up_mm(t):
            # ups 4-7 reuse the score/transpose banks (dead by then) so the
            # 2-deep "ua" ring doesn't serialize them behind cols t0-3.
            tag = "ua" if t < 4 else ("s0" if t < 6 else "tp")
            up = ps.tile([P, D], f32, tag=tag)
            ups[t] = up
            nc.tensor.matmul(
                up[:], ET16[:, ds(t * P, P)], u16[:], start=True, stop=True
            )

        def col1(t):
            if t % 2 == 0:
                nc.scalar.mul(stgA[:, ds(t * D, D)], ups[t][:], z_rec[:, ds(t, 1)])
            else:
                nc.vector.tensor_scalar_mul(
                    stgA[:, ds(t * D, D)], ups[t][:], z_rec[:, ds(t, 1)]
                )

        def col2_pair_gps(t0):
            nc.gpsimd.tensor_mul(
                stgB[:, ds(t0 * D, 2 * D)],
                stgA[:, ds(t0 * D, 2 * D)],
                h_f[:, ds(t0 * D, 2 * D)],
            )

        def col2(t):
            nc.vector.scalar_tensor_tensor(
                stgB[:, ds(t * D, D)], ups[t][:], z_rec[:, ds(t, 1)],
                h_f[:, ds(t * D, D)], op0=MUL, op1=MUL,
            )

        def stg_writes(half, nt=TPB):
            # stgA pieces issue on sync, stgB on the scalar HWDGE queue so
            # the tail write issues (~0.65us each) don't serialize.
            for t0 in range(half * TPB, (half + 1) * TPB, nt):
                for col, src, q in ((0, stgA, nc.sync), (1, stgB, nc.scalar)):
                    q.dma_start(
                        out12_d[ds(t0 * P, nt * P), ds(col * D, D)]
                        .rearrange("(t p) c -> p t c", p=P),
                        src[:, ds(t0 * D, nt * D)]
                        .rearrange("p (t c) -> p t c", t=nt),
                    )

        # ---- spine ----
        up_mm(0)
        up_mm(1)
        block_reduce(1)
        up_mm(2)
        up_mm(3)
        hap_block(0)
        hap_block(1)

        # q2c chain: rzq = 1/sum_i m_exp[i] folds into the hap eviction
        # (ha_row = normalized h_a); haT = per-chunk column form of h_a so
        # o4T = hT16 * haT[k] is a per-partition tensor_scalar in bf16.
        zqp = ps.tile([1, 1], f32, tag="acc", bufs=1)
        nc.tensor.matmul(zqp[:], mrow[:], ones_col[:], start=True, stop=True)
        rzq = consts.tile([1, 1], f32)
        nc.vector.reciprocal(rzq[:], zqp[:])
        up_mm(4)
        up_mm(5)
        up_mm(6)
        up_mm(7)
        # ha_row is the UNNORMALIZED hap evict (no rzq wait); 1/zq is
        # partition-broadcast by a K=1 matmul into a dead tp bank and folded
        # into the tiny haT evicts instead.
        ha_row = consts.tile([1, D], bf16)
        nc.scalar.copy(ha_row[:], hap[:])
        rzq_bc = ps.tile([P, 1], f32, tag="hap", bufs=1)
        nc.tensor.matmul(rzq_bc[:], ones_row_f[:], rzq[:], start=True, stop=True)
        # bf16 PSUM writes must stay 4-byte aligned -> pad each haT column
        # to a 2-element stride and gather on the evict.
        haT_p = ps.tile([P, 2 * KC], bf16, tag="acc", bufs=1)
        for k in range(KC):
            nc.tensor.transpose(
                haT_p[:, ds(2 * k, 1)], ha_row[:, ds(k * P, P)],
                ident16[ds(0, 1), ds(0, 1)],
            )
        haT_strided = haT_p[:].rearrange("p (k two) -> p k two", two=2)[:, :, 0]
        haT = consts.tile([P, KC], f32)
        nc.vector.tensor_scalar_mul(haT[:], haT_strided, rzq_bc[:])
        haT16 = consts.tile([P, KC], bf16)
        nc.vector.tensor_scalar_mul(haT16[:], haT_strided, rzq_bc[:])

        o4T16 = consts.tile([P, KC * JX], bf16)

        def o4T_chunk(k, eng):
            if eng is nc.scalar:
                # InstActivation Copy+scale: per-partition scale, fast path
                nc.scalar.mul(o4T16[:, ds(k * JX, JX)], hT3[:, k, :],
                              haT[:, ds(k, 1)])
            else:
                # all-bf16 TT with a stride-0 broadcast scalar: 2x fast path
                # (TensorScalarPtr with bf16 SBUF in0 is a ~15us ucode path)
                eng.tensor_mul(
                    o4T16[:, ds(k * JX, JX)].rearrange("p (o x) -> p o x", o=1),
                    hT3[:, k, :].rearrange("p (o x) -> p o x", o=1),
                    haT16[:, ds(k, 1)].rearrange("p (o c) -> p o c", o=1)
                    .broadcast_to([P, 1, JX]),
                )

        def o4T_write(k0, nk=2):
            for k in range(k0, k0 + nk):
                q = nc.sync if k % 2 == 0 else nc.scalar
                q.dma_start(
                    o4T_d[ds(k * P, P), :],
                    o4T16[:, ds(k * JX, JX)],
                )

        col1(0)
        col1(1)
        col2_pair_gps(0)
        col1(2)
        col1(3)
        col2(2)
        col2(3)
        o4T_chunk(0, nc.scalar)
        o4T_chunk(1, nc.gpsimd)
        col1(4)
        col1(5)
        col2(4)
        col2(5)
        stg_writes(0)
        o4T_chunk(2, nc.scalar)
        o4T_write(0)
        col1(6)
        col1(7)
        o4T_chunk(3, nc.gpsimd)
        col2(6)
        col2(7)
        o4T_write(2, nk=1)
        stg_writes(1, nt=2)
        o4T_write(3, nk=1)

    nc.compile()
    return nc


def _get_nc():
    if "nc" not in _CACHE:
        _CACHE["nc"] = _build_program()
    return _CACHE["nc"]


def _ensure_axon_hooks_stub():
    # concourse imports antenv.axon_hooks when tracing is requested via env;
    # provide a no-op stub if the image lacks it so runs degrade gracefully.
    import sys
    import types

    try:
        import antenv.axon_hooks  # noqa: F401
    except ImportError:
        mod = types.ModuleType("antenv.axon_hooks")
        _hook = [None]
        mod.set_axon_ntff_profile_hook = lambda hook: _hook.__setitem__(0, hook)
        mod.get_axon_ntff_profile_hook = lambda: _hook[0]
        sys.modules["antenv.axon_hooks"] = mod


def _postprocess(res):
    out = np.empty((N_B, JX, 4 * D), dtype=np.float32)
    for n in range(N_B):
        r = res.results[n]
        out[n, :, :D] = np.asarray(r["out0"])
        out[n, :, D:3 * D] = np.asarray(r["out12"]).astype(np.float32)
        out[n, :, 3 * D:] = np.asarray(r["o4T"]).astype(np.float32).T
    return out.reshape(N_B, M_B, JX, 4 * D)


def kernel(h, u, alpha_w, alpha_b=None, **_unused):
    _ensure_axon_hooks_stub()
    from concourse.bass_utils import run_bass_kernel_spmd

    h = np.ascontiguousarray(np.asarray(h, dtype=np.float32)).reshape(N_B, JX, D)
    u = np.ascontiguousarray(np.asarray(u, dtype=np.float32)).reshape(N_B, JQ, D)
    alpha_w = np.ascontiguousarray(np.asarray(alpha_w, dtype=np.float32)).reshape(3 * D)

    nc = _get_nc()
    in_maps = [
        {"h": h[n], "u": u[n], "alpha_w": alpha_w} for n in range(N_B)
    ]
    res = run_bass_kernel_spmd(nc, in_maps, core_ids=list(range(N_B)))
    return _postprocess(res)
